# revision 1
# baseline (speedup 1.0000x reference)
import numpy as np
import ml_dtypes

import concourse.bacc as bacc
import concourse.tile as tile
from concourse import mybir

# Problem: NIMSCrossEntropyLoss
#   preds (4, 4, 4, 512, 512) f32, targets (4, 4, 512, 512) int32
#   Only the S=-1 slice contributes:
#   loss = [sum_pixels logsumexp_c(p) - sum_pixels p[target]] / N_BATCH
# Shard the 4*512*512 = 1048576 pixels over 8 cores:
#   131072 pixels/core as [128 partitions, 1024 free] channel planes (bf16).
# v3: per-plane DRAM tensors + 3 parallel DMA queues (ACT/SP/SWDGE) +
#     per-plane exp and a DVE order that feeds ln as early as possible.

N_CORES = 8
P = 128           # partitions
C = 4             # classes
N_BATCH = 4       # reference divides by this
F = 1024          # pixels per partition per core

BF16 = mybir.dt.bfloat16
F32 = mybir.dt.float32

_PATCHED = False


def _patch_act_tables():
    """Force exp+ln into the combined ACT table so only one table load is
    emitted (greedy per-function set choice otherwise alternates sets)."""
    global _PATCHED
    if _PATCHED:
        return
    import concourse.hw_specs as hw_specs
    real = hw_specs.get_activation_tables
    Exp = mybir.ActivationFunctionType.Exp
    Ln = mybir.ActivationFunctionType.Ln

    def patched(arch):
        out = {}
        for name, fns in dict(real(arch)).items():
            if name != "natural_log_exp_and_others":
                fns = fns - {Exp, Ln}
            out[name] = fns
        return out

    bacc.get_activation_tables = patched
    _PATCHED = True


def build_nc(f=F, finalize=True):
    """One core's shard: p0..p3 channel planes [P, f] bf16, tgt [P, f] bf16;
    out [P, 5] f32 = per-partition sums (p_t for c=0..3, lse)."""
    _patch_act_tables()
    nc = bacc.Bacc("TRN2", target_bir_lowering=False, debug=False)
    planes = [nc.dram_tensor(f"p{c}", (P, f), BF16, kind="ExternalInput").ap()
              for c in range(C)]
    tgt = nc.dram_tensor("tgt", (P, f), BF16, kind="ExternalInput").ap()
    out = nc.dram_tensor("out", (P, 5), F32, kind="ExternalOutput").ap()

    Exp = mybir.ActivationFunctionType.Exp
    Ln = mybir.ActivationFunctionType.Ln

    with tile.TileContext(nc) as tc:
        with tc.tile_pool(name="w", bufs=1) as w:
            pt = [w.tile([P, f], BF16, name=f"pt{c}") for c in range(C)]
            tt = w.tile([P, f], BF16)

            # Sync + GpSimd DMA queues only: scalar.dma_start forces a
            # spurious extra ACT table load whose DRAM traffic starves the
            # input DMAs. Interleaved completion -> p0, tgt, p1, p2, p3.
            # (Splitting tgt/p0 into half-transfers was tried and is slower:
            # extra issue overhead pushes the ACT table load late, and a
            # concurrent gpsimd add causes SBUF contention that slows DVE.)
            nc.sync.dma_start(out=pt[0], in_=planes[0])
            nc.gpsimd.dma_start(out=tt, in_=tgt)
            nc.sync.dma_start(out=pt[1], in_=planes[1])
            nc.gpsimd.dma_start(out=pt[2], in_=planes[2])
            nc.sync.dma_start(out=pt[3], in_=planes[3])

            res = w.tile([P, 5], F32)
            e = [w.tile([P, f], BF16, name=f"e{c}") for c in range(C)]
            for c in range(C):
                nc.scalar.activation(out=e[c], in_=pt[c], func=Exp)

            scr = w.tile([P, 4 * f], BF16)

            def stt(c):
                nc.vector.scalar_tensor_tensor(
                    out=scr[:, c * f:(c + 1) * f], in0=tt, scalar=float(c),
                    in1=pt[c],
                    op0=mybir.AluOpType.is_equal, op1=mybir.AluOpType.mult,
                    accum_out=res[:, c:c + 1],
                )

            s01 = w.tile([P, f], BF16)
            s012 = w.tile([P, f], BF16)
            s = w.tile([P, f], BF16)

            # The scheduler batches all 4 stts first on DVE regardless of
            # emission order (priority hints don't change it), then runs the
            # three adds and ln.
            stt(0)
            stt(1)
            nc.vector.tensor_tensor(out=s01, in0=e[0], in1=e[1],
                                    op=mybir.AluOpType.add)
            stt(2)
            nc.vector.tensor_tensor(out=s012, in0=s01, in1=e[2],
                                    op=mybir.AluOpType.add)
            nc.vector.tensor_tensor(out=s, in0=s012, in1=e[3],
                                    op=mybir.AluOpType.add)
            stt(3)

            lnout = w.tile([P, f], BF16)
            nc.scalar.activation(out=lnout, in_=s, func=Ln,
                                 accum_out=res[:, 4:5])

            nc.sync.dma_start(out=out, in_=res)
    if finalize:
        nc.finalize()
    return nc


_NC_CACHE = {}


def _get_nc(f=F):
    if f not in _NC_CACHE:
        _NC_CACHE[f] = build_nc(f)
    return _NC_CACHE[f]


def prep_inputs(preds, targets):
    """Host-side shard prep: S=-1 slice, per-channel planes, 8-way split."""
    p = np.asarray(preds)[:, -1]       # (N=4, C=4, 512, 512) f32
    t = np.asarray(targets)[:, -1]     # (4, 512, 512) int
    arr = np.transpose(p, (1, 0, 2, 3)).reshape(C, N_CORES, P, -1)
    arr = arr.astype(ml_dtypes.bfloat16)
    tf = t.reshape(N_CORES, P, -1).astype(ml_dtypes.bfloat16)
    maps = []
    for k in range(N_CORES):
        m = {f"p{c}": np.ascontiguousarray(arr[c, k]) for c in range(C)}
        m["tgt"] = tf[k]
        maps.append(m)
    return maps


def reduce_outputs(results):
    total = 0.0
    for d in results:
        o = d["out"].astype(np.float64)
        total += float(o[:, 4].sum() - o[:, 0:4].sum())
    return np.float32(total / N_BATCH)


def kernel(preds, targets, _trace=False, _trace_kwargs=None):
    from concourse.bass_utils import run_bass_kernel_spmd

    in_maps = prep_inputs(preds, targets)
    f = in_maps[0]["tgt"].shape[1]
    nc = _get_nc(f=f)
    r = run_bass_kernel_spmd(
        nc, in_maps, core_ids=list(range(N_CORES)),
        trace=_trace, **(_trace_kwargs or {}),
    )
    kernel.last_run = r
    return reduce_outputs(r.results)


kernel.last_run = None



# revision 2
# speedup vs baseline: 1.0862x; 1.0862x over previous
import numpy as np
import ml_dtypes

import concourse.bacc as bacc
import concourse.tile as tile
from concourse import mybir

# Problem: NIMSCrossEntropyLoss
#   preds (4, 4, 4, 512, 512) f32, targets (4, 4, 512, 512) int
#   Only the S=-1 slice contributes:
#   loss = [sum_pixels logsumexp_c(p) - sum_pixels p[target]] / N_BATCH
# Shard the 4*512*512 = 1048576 pixels over 8 cores:
#   131072 pixels/core as [128 partitions, 1024 free] channel planes (bf16).
# v4: sum of exp-planes moved off DVE onto the idle PE via identity-matmul
#     accumulation into PSUM; Ln reads PSUM directly. DVE only runs the 4
#     STT gather ops (off the ACT critical path).

N_CORES = 8
P = 128           # partitions
C = 4             # classes
N_BATCH = 4       # reference divides by this
F = 1024          # pixels per partition per core

BF16 = mybir.dt.bfloat16
F32 = mybir.dt.float32

_PATCHED = False


def _patch_act_tables():
    """Force exp+ln into the combined ACT table so only one table load is
    emitted (greedy per-function set choice otherwise alternates sets)."""
    global _PATCHED
    if _PATCHED:
        return
    import concourse.hw_specs as hw_specs
    real = hw_specs.get_activation_tables
    Exp = mybir.ActivationFunctionType.Exp
    Ln = mybir.ActivationFunctionType.Ln

    def patched(arch):
        out = {}
        for name, fns in dict(real(arch)).items():
            if name != "natural_log_exp_and_others":
                fns = fns - {Exp, Ln}
            out[name] = fns
        return out

    bacc.get_activation_tables = patched
    _PATCHED = True


def build_nc(f=F, finalize=True):
    """One core's shard: p0..p3 channel planes [P, f] bf16, tgt [P, f] bf16,
    eye [P, P] bf16; out [P, 5] f32 = per-partition sums (p_t for c=0..3, lse)."""
    _patch_act_tables()
    nc = bacc.Bacc("TRN2", target_bir_lowering=False, debug=False)
    planes = [nc.dram_tensor(f"p{c}", (P, f), BF16, kind="ExternalInput").ap()
              for c in range(C)]
    tgt = nc.dram_tensor("tgt", (P, f), BF16, kind="ExternalInput").ap()
    eye = nc.dram_tensor("eye", (P, P), BF16, kind="ExternalInput").ap()
    out = nc.dram_tensor("out", (P, 5), F32, kind="ExternalOutput").ap()

    Exp = mybir.ActivationFunctionType.Exp
    Ln = mybir.ActivationFunctionType.Ln
    h = f // 2  # PSUM bank half (512 f32 = one 2KB bank)

    with tile.TileContext(nc) as tc:
        with tc.tile_pool(name="w", bufs=1) as w, \
             tc.tile_pool(name="ps", bufs=1, space="PSUM") as pp:
            pt = [w.tile([P, f], BF16, name=f"pt{c}") for c in range(C)]
            tt = w.tile([P, f], BF16)
            te = w.tile([P, P], BF16)

            # DMA order: p-planes sequential on the sync HWDGE queue so the
            # first plane gets full HBM bandwidth and the ACT exp chain can
            # start as early as possible; eye+tgt ride the gpsimd queue.
            nc.sync.dma_start(out=pt[0], in_=planes[0])
            nc.gpsimd.dma_start(out=te, in_=eye)
            nc.sync.dma_start(out=pt[1], in_=planes[1])
            nc.gpsimd.dma_start(out=tt, in_=tgt)
            nc.sync.dma_start(out=pt[2], in_=planes[2])
            nc.sync.dma_start(out=pt[3], in_=planes[3])

            res = w.tile([P, 5], F32)
            e = [w.tile([P, f], BF16, name=f"e{c}") for c in range(C)]
            psum = pp.tile([P, f], F32)
            scr = w.tile([P, 4 * f], BF16)

            def stt(c):
                nc.vector.scalar_tensor_tensor(
                    out=scr[:, c * f:(c + 1) * f], in0=tt, scalar=float(c),
                    in1=pt[c],
                    op0=mybir.AluOpType.is_equal, op1=mybir.AluOpType.mult,
                    accum_out=res[:, c:c + 1],
                )

            for c in range(C):
                nc.scalar.activation(out=e[c], in_=pt[c], func=Exp)
                # Accumulate e_c into PSUM on the (idle) PE: out = eye.T @ e_c
                # adds e_c elementwise. One matmul per 2KB PSUM bank half.
                nc.tensor.matmul(psum[:, 0:h], te, e[c][:, 0:h],
                                 start=(c == 0), stop=(c == C - 1))
                nc.tensor.matmul(psum[:, h:f], te, e[c][:, h:f],
                                 start=(c == 0), stop=(c == C - 1))
                stt(c)

            lnout = w.tile([P, f], BF16)
            nc.scalar.activation(out=lnout, in_=psum, func=Ln,
                                 accum_out=res[:, 4:5])

            nc.sync.dma_start(out=out, in_=res)
    if finalize:
        nc.finalize()
    return nc


_NC_CACHE = {}


def _get_nc(f=F):
    if f not in _NC_CACHE:
        _NC_CACHE[f] = build_nc(f)
    return _NC_CACHE[f]


def prep_inputs(preds, targets):
    """Host-side shard prep: S=-1 slice, per-channel planes, 8-way split."""
    p = np.asarray(preds)[:, -1]       # (N=4, C=4, 512, 512) f32
    t = np.asarray(targets)[:, -1]     # (4, 512, 512) int
    arr = np.transpose(p, (1, 0, 2, 3)).reshape(C, N_CORES, P, -1)
    arr = arr.astype(ml_dtypes.bfloat16)
    tf = t.reshape(N_CORES, P, -1).astype(ml_dtypes.bfloat16)
    ident = np.eye(P, dtype=ml_dtypes.bfloat16)
    maps = []
    for k in range(N_CORES):
        m = {f"p{c}": np.ascontiguousarray(arr[c, k]) for c in range(C)}
        m["tgt"] = tf[k]
        m["eye"] = ident
        maps.append(m)
    return maps


def reduce_outputs(results):
    total = 0.0
    for d in results:
        o = d["out"].astype(np.float64)
        total += float(o[:, 4].sum() - o[:, 0:4].sum())
    return np.float32(total / N_BATCH)


def kernel(preds, targets, _trace=False, _trace_kwargs=None):
    from concourse.bass_utils import run_bass_kernel_spmd

    in_maps = prep_inputs(preds, targets)
    f = in_maps[0]["tgt"].shape[1]
    nc = _get_nc(f=f)
    r = run_bass_kernel_spmd(
        nc, in_maps, core_ids=list(range(N_CORES)),
        trace=_trace, **(_trace_kwargs or {}),
    )
    kernel.last_run = r
    return reduce_outputs(r.results)


kernel.last_run = None


# revision 5
# speedup vs baseline: 1.1572x; 1.0653x over previous
import numpy as np
import ml_dtypes
from contextlib import ExitStack

import concourse.bacc as bacc
from concourse import mybir

# Problem: NIMSCrossEntropyLoss
#   preds (4, 4, 4, 512, 512) f32, targets (4, 4, 512, 512) int
#   Only the S=-1 slice contributes:
#   loss = [sum_pixels logsumexp_c(p) - sum_pixels p[target]] / N_BATCH
# Shard the 4*512*512 = 1048576 pixels over 8 cores:
#   131072 pixels/core as [128 partitions, 1024 free] channel planes (bf16).
# v5: raw bacc (no TileContext) — input DMAs and the ACT table load issue
#     right after the NEFF engine preamble instead of behind Tile's entry
#     barrier; sum of exp-planes accumulates on the idle PE via identity
#     matmuls into PSUM; Ln reads PSUM directly; DVE runs only the 4 STT
#     gather ops. All cross-engine deps via explicit semaphores.

N_CORES = 8
P = 128           # partitions
C = 4             # classes
N_BATCH = 4       # reference divides by this
F = 1024          # pixels per partition per core

BF16 = mybir.dt.bfloat16
F32 = mybir.dt.float32

_PATCHED = False


def _patch_act_tables():
    """Force exp+ln into the combined ACT table so only one table load is
    emitted (greedy per-function set choice otherwise alternates sets)."""
    global _PATCHED
    if _PATCHED:
        return
    import concourse.hw_specs as hw_specs
    real = hw_specs.get_activation_tables
    Exp = mybir.ActivationFunctionType.Exp
    Ln = mybir.ActivationFunctionType.Ln

    def patched(arch):
        out = {}
        for name, fns in dict(real(arch)).items():
            if name != "natural_log_exp_and_others":
                fns = fns - {Exp, Ln}
            out[name] = fns
        return out

    bacc.get_activation_tables = patched
    _PATCHED = True


def build_nc(f=F, finalize=True):
    """One core's shard: p0..p3 channel planes [P, f] bf16, tgt [P, f] bf16,
    eye [P, P] bf16; out [P, 5] f32 = per-partition sums (p_t for c=0..3, lse)."""
    _patch_act_tables()
    nc = bacc.Bacc("TRN2", target_bir_lowering=False, debug=False)
    planes = [nc.dram_tensor(f"p{c}", (P, f), BF16, kind="ExternalInput").ap()
              for c in range(C)]
    tgt = nc.dram_tensor("tgt", (P, f), BF16, kind="ExternalInput").ap()
    eye = nc.dram_tensor("eye", (P, P), BF16, kind="ExternalInput").ap()
    outd = nc.dram_tensor("out", (P, 5), F32, kind="ExternalOutput").ap()

    Exp = mybir.ActivationFunctionType.Exp
    Ln = mybir.ActivationFunctionType.Ln
    h = f // 2  # PSUM bank half (512 f32 = one 2KB bank)

    es = ExitStack()
    sb = lambda name, shape, dt: es.enter_context(
        nc.sbuf_tensor(name, shape, dt)).ap()
    with nc.Block(name="ce") as block:
        s_p = [es.enter_context(nc.semaphore(f"s_p{c}")) for c in range(C)]
        s_eye = es.enter_context(nc.semaphore("s_eye"))
        s_tgt = es.enter_context(nc.semaphore("s_tgt"))
        s_e = es.enter_context(nc.semaphore("s_e"))
        s_mm = es.enter_context(nc.semaphore("s_mm"))
        s_res = es.enter_context(nc.semaphore("s_res"))
        s_out = es.enter_context(nc.semaphore("s_out"))

        pt = [sb(f"pt{c}", [P, f], BF16) for c in range(C)]
        tt = sb("tt", [P, f], BF16)
        te = sb("te", [P, P], BF16)
        e = [sb(f"e{c}", [P, f], BF16) for c in range(C)]
        scr = sb("scr", [P, 4 * f], BF16)
        lnout = sb("lnout", [P, f], BF16)
        res = sb("res", [P, 5], F32)
        dmy = sb("dmy", [P, 1], BF16)
        psum = es.enter_context(nc.psum_tensor("ps", [P, f], F32)).ap()

        @block.sync
        def _(sync):
            # p-planes sequential on the HWDGE queue: first plane gets full
            # HBM bandwidth so the ACT exp chain starts as early as possible.
            for c in range(C):
                sync.dma_start(out=pt[c], in_=planes[c]).then_inc(s_p[c], 16)
            sync.wait_ge(s_res, 5)  # 4 STT accums + ln accum
            sync.dma_start(out=outd, in_=res).then_inc(s_out, 16)

        @block.gpsimd
        def _(gpsimd):
            gpsimd.dma_start(out=te, in_=eye).then_inc(s_eye, 16)
            gpsimd.dma_start(out=tt, in_=tgt).then_inc(s_tgt, 16)

        @block.scalar
        def _(scalar):
            # Dummy activation first: the act-table-load pass places the
            # (1.3us) table DMA before it, so the table streams in parallel
            # with the input DMAs instead of serializing before exp0.
            scalar.activation(out=dmy, in_=dmy, func=Exp)
            for c in range(C):
                scalar.wait_ge(s_p[c], 16)
                scalar.activation(out=e[c], in_=pt[c], func=Exp).then_inc(s_e, 1)
            scalar.wait_ge(s_mm, 2)
            scalar.activation(out=lnout, in_=psum, func=Ln,
                              accum_out=res[:, 4:5]).then_inc(s_res, 1)

        @block.tensor
        def _(tensor):
            tensor.wait_ge(s_eye, 16)
            for c in range(C):
                tensor.wait_ge(s_e, c + 1)
                m0 = tensor.matmul(psum[:, 0:h], te, e[c][:, 0:h],
                                   start=(c == 0), stop=(c == C - 1))
                m1 = tensor.matmul(psum[:, h:f], te, e[c][:, h:f],
                                   start=(c == 0), stop=(c == C - 1))
                if c == C - 1:
                    m0.then_inc(s_mm, 1)
                    m1.then_inc(s_mm, 1)

        @block.vector
        def _(vector):
            vector.wait_ge(s_tgt, 16)
            for c in range(C):
                vector.wait_ge(s_p[c], 16)
                vector.scalar_tensor_tensor(
                    out=scr[:, c * f:(c + 1) * f], in0=tt, scalar=float(c),
                    in1=pt[c],
                    op0=mybir.AluOpType.is_equal, op1=mybir.AluOpType.mult,
                    accum_out=res[:, c:c + 1],
                ).then_inc(s_res, 1)

    es.close()
    if finalize:
        nc.finalize()
    return nc


_NC_CACHE = {}


def _get_nc(f=F):
    if f not in _NC_CACHE:
        _NC_CACHE[f] = build_nc(f)
    return _NC_CACHE[f]


def prep_inputs(preds, targets):
    """Host-side shard prep: S=-1 slice, per-channel planes, 8-way split."""
    p = np.asarray(preds)[:, -1]       # (N=4, C=4, 512, 512) f32
    t = np.asarray(targets)[:, -1]     # (4, 512, 512) int
    arr = np.transpose(p, (1, 0, 2, 3)).reshape(C, N_CORES, P, -1)
    arr = arr.astype(ml_dtypes.bfloat16)
    tf = t.reshape(N_CORES, P, -1).astype(ml_dtypes.bfloat16)
    ident = np.eye(P, dtype=ml_dtypes.bfloat16)
    maps = []
    for k in range(N_CORES):
        m = {f"p{c}": np.ascontiguousarray(arr[c, k]) for c in range(C)}
        m["tgt"] = tf[k]
        m["eye"] = ident
        maps.append(m)
    return maps


def reduce_outputs(results):
    total = 0.0
    for d in results:
        o = d["out"].astype(np.float64)
        total += float(o[:, 4].sum() - o[:, 0:4].sum())
    return np.float32(total / N_BATCH)


def kernel(preds, targets, _trace=False, _trace_kwargs=None):
    from concourse.bass_utils import run_bass_kernel_spmd

    in_maps = prep_inputs(preds, targets)
    f = in_maps[0]["tgt"].shape[1]
    nc = _get_nc(f=f)
    r = run_bass_kernel_spmd(
        nc, in_maps, core_ids=list(range(N_CORES)),
        trace=_trace, **(_trace_kwargs or {}),
    )
    kernel.last_run = r
    return reduce_outputs(r.results)


kernel.last_run = None


# revision 6
# speedup vs baseline: 1.2115x; 1.0469x over previous
import numpy as np
import ml_dtypes
from contextlib import ExitStack

import concourse.bacc as bacc
from concourse import mybir

# Problem: NIMSCrossEntropyLoss
#   preds (4, 4, 4, 512, 512) f32, targets (4, 4, 512, 512) int
#   Only the S=-1 slice contributes:
#   loss = [sum_pixels logsumexp_c(p) - sum_pixels p[target]] / N_BATCH
# Shard the 4*512*512 = 1048576 pixels over 8 cores:
#   131072 pixels/core as [128 partitions, 1024 free] channel planes (bf16).
# v6: raw bacc (no TileContext). Plane-pair DMA transfers (512KB each)
#     amortize the ~1.5us per-transfer completion latency; exp runs as two
#     merged [128,2048] ACTIVATEs; the exp-plane sum accumulates on the idle
#     PE via identity matmuls into PSUM; Ln reads PSUM directly; DVE runs
#     only the 4 STT gather ops. All cross-engine deps via explicit sems.

N_CORES = 8
P = 128           # partitions
C = 4             # classes
N_BATCH = 4       # reference divides by this
F = 1024          # pixels per partition per core

BF16 = mybir.dt.bfloat16
F32 = mybir.dt.float32

_PATCHED = False


def _patch_act_tables():
    """Force exp+ln into the combined ACT table so only one table load is
    emitted (greedy per-function set choice otherwise alternates sets)."""
    global _PATCHED
    if _PATCHED:
        return
    import concourse.hw_specs as hw_specs
    real = hw_specs.get_activation_tables
    Exp = mybir.ActivationFunctionType.Exp
    Ln = mybir.ActivationFunctionType.Ln

    def patched(arch):
        out = {}
        for name, fns in dict(real(arch)).items():
            if name != "natural_log_exp_and_others":
                fns = fns - {Exp, Ln}
            out[name] = fns
        return out

    bacc.get_activation_tables = patched
    _PATCHED = True


def build_nc(f=F, finalize=True):
    """One core's shard: p01/p23 plane-pair tensors [P, 2f] bf16 (planes
    concatenated on the free axis), tgt [P, f] bf16, eye [P, P] bf16;
    out [P, 5] f32 = per-partition sums (p_t for c=0..3, lse)."""
    _patch_act_tables()
    nc = bacc.Bacc("TRN2", target_bir_lowering=False, debug=False)
    p01 = nc.dram_tensor("p01", (P, 2 * f), BF16, kind="ExternalInput").ap()
    p23 = nc.dram_tensor("p23", (P, 2 * f), BF16, kind="ExternalInput").ap()
    tgt = nc.dram_tensor("tgt", (P, f), BF16, kind="ExternalInput").ap()
    eye = nc.dram_tensor("eye", (P, P), BF16, kind="ExternalInput").ap()
    outd = nc.dram_tensor("out", (P, 5), F32, kind="ExternalOutput").ap()

    Exp = mybir.ActivationFunctionType.Exp
    Ln = mybir.ActivationFunctionType.Ln
    h = f // 2  # PSUM bank half (512 f32 = one 2KB bank)

    es = ExitStack()
    sb = lambda name, shape, dt: es.enter_context(
        nc.sbuf_tensor(name, shape, dt)).ap()
    with nc.Block(name="ce") as block:
        s_p = [es.enter_context(nc.semaphore(f"s_p{i}")) for i in range(2)]
        s_eye = es.enter_context(nc.semaphore("s_eye"))
        s_tgt = es.enter_context(nc.semaphore("s_tgt"))
        s_e = es.enter_context(nc.semaphore("s_e"))
        s_mm = es.enter_context(nc.semaphore("s_mm"))
        s_res = es.enter_context(nc.semaphore("s_res"))
        s_out = es.enter_context(nc.semaphore("s_out"))

        pt = [sb(f"pt{i}", [P, 2 * f], BF16) for i in range(2)]
        tt = sb("tt", [P, f], BF16)
        te = sb("te", [P, P], BF16)
        e = [sb(f"e{i}", [P, 2 * f], BF16) for i in range(2)]
        scr = sb("scr", [P, 4 * f], BF16)
        lnout = sb("lnout", [P, f], BF16)
        res = sb("res", [P, 5], F32)
        dmy = sb("dmy", [P, 1], BF16)
        psum = es.enter_context(nc.psum_tensor("ps", [P, f], F32)).ap()

        @block.sync
        def _(sync):
            # Two plane-pair transfers on the HWDGE queue: first pair gets
            # full HBM bandwidth so the exp chain starts as early as possible.
            sync.dma_start(out=pt[0], in_=p01).then_inc(s_p[0], 16)
            sync.dma_start(out=pt[1], in_=p23).then_inc(s_p[1], 16)
            sync.wait_ge(s_res, 5)  # 4 STT accums + ln accum
            sync.dma_start(out=outd, in_=res).then_inc(s_out, 16)

        @block.gpsimd
        def _(gpsimd):
            gpsimd.dma_start(out=te, in_=eye).then_inc(s_eye, 16)
            gpsimd.dma_start(out=tt, in_=tgt).then_inc(s_tgt, 16)

        @block.scalar
        def _(scalar):
            # Dummy activation first: the act-table-load pass places the
            # (1.3us) table DMA before it, so the table streams in parallel
            # with the input DMAs instead of serializing before exp0.
            scalar.activation(out=dmy, in_=dmy, func=Exp)
            for i in range(2):
                scalar.wait_ge(s_p[i], 16)
                scalar.activation(out=e[i], in_=pt[i], func=Exp).then_inc(s_e, 1)
            scalar.wait_ge(s_mm, 2)
            scalar.activation(out=lnout, in_=psum, func=Ln,
                              accum_out=res[:, 4:5]).then_inc(s_res, 1)

        @block.tensor
        def _(tensor):
            tensor.wait_ge(s_eye, 16)
            for i in range(2):
                tensor.wait_ge(s_e, i + 1)
                for half in range(2):
                    lo = half * h
                    for cc in range(2):   # plane within the pair
                        m = tensor.matmul(
                            psum[:, lo:lo + h], te,
                            e[i][:, cc * f + lo:cc * f + lo + h],
                            start=(i == 0 and cc == 0),
                            stop=(i == 1 and cc == 1))
                        if i == 1 and cc == 1:
                            m.then_inc(s_mm, 1)

        @block.vector
        def _(vector):
            vector.wait_ge(s_tgt, 16)
            for c in range(C):
                vector.wait_ge(s_p[c // 2], 16)
                vector.scalar_tensor_tensor(
                    out=scr[:, c * f:(c + 1) * f], in0=tt, scalar=float(c),
                    in1=pt[c // 2][:, (c % 2) * f:(c % 2) * f + f],
                    op0=mybir.AluOpType.is_equal, op1=mybir.AluOpType.mult,
                    accum_out=res[:, c:c + 1],
                ).then_inc(s_res, 1)

    es.close()
    if finalize:
        nc.finalize()
    return nc


_NC_CACHE = {}


def _get_nc(f=F):
    if f not in _NC_CACHE:
        _NC_CACHE[f] = build_nc(f)
    return _NC_CACHE[f]


def prep_inputs(preds, targets):
    """Host-side shard prep: S=-1 slice, per-channel planes, 8-way split."""
    p = np.asarray(preds)[:, -1]       # (N=4, C=4, 512, 512) f32
    t = np.asarray(targets)[:, -1]     # (4, 512, 512) int
    arr = np.transpose(p, (1, 0, 2, 3)).reshape(C, N_CORES, P, -1)
    arr = arr.astype(ml_dtypes.bfloat16)
    tf = t.reshape(N_CORES, P, -1).astype(ml_dtypes.bfloat16)
    ident = np.eye(P, dtype=ml_dtypes.bfloat16)
    maps = []
    for k in range(N_CORES):
        m = {
            "p01": np.ascontiguousarray(
                np.concatenate([arr[0, k], arr[1, k]], axis=1)),
            "p23": np.ascontiguousarray(
                np.concatenate([arr[2, k], arr[3, k]], axis=1)),
            "tgt": tf[k],
            "eye": ident,
        }
        maps.append(m)
    return maps


def reduce_outputs(results):
    total = 0.0
    for d in results:
        o = d["out"].astype(np.float64)
        total += float(o[:, 4].sum() - o[:, 0:4].sum())
    return np.float32(total / N_BATCH)


def kernel(preds, targets, _trace=False, _trace_kwargs=None):
    from concourse.bass_utils import run_bass_kernel_spmd

    in_maps = prep_inputs(preds, targets)
    f = in_maps[0]["tgt"].shape[1]
    nc = _get_nc(f=f)
    r = run_bass_kernel_spmd(
        nc, in_maps, core_ids=list(range(N_CORES)),
        trace=_trace, **(_trace_kwargs or {}),
    )
    kernel.last_run = r
    return reduce_outputs(r.results)


kernel.last_run = None


# revision 7
# speedup vs baseline: 1.2389x; 1.0226x over previous
import numpy as np
import ml_dtypes
from contextlib import ExitStack

import concourse.bacc as bacc
from concourse import mybir

# Problem: NIMSCrossEntropyLoss
#   preds (4, 4, 4, 512, 512) f32, targets (4, 4, 512, 512) int
#   Only the S=-1 slice contributes:
#   loss = [sum_pixels logsumexp_c(p) - sum_pixels p[target]] / N_BATCH
# Shard the 4*512*512 = 1048576 pixels over 8 cores:
#   131072 pixels/core as [128 partitions, 1024 free] channel planes (bf16).
# v7: raw bacc. Plane-pair transfers ride BOTH DMA queues concurrently
#     (sync: p01+tgt, gpsimd: p23+eye); exp as two merged [128,2048]
#     ACTIVATEs; exp-plane sum accumulates on the idle PE via identity
#     matmuls into PSUM (per-bank groups); Ln split into two half-plane
#     ACTIVATEs so the first overlaps the second half's matmuls.

N_CORES = 8
P = 128           # partitions
C = 4             # classes
N_BATCH = 4       # reference divides by this
F = 1024          # pixels per partition per core

BF16 = mybir.dt.bfloat16
F32 = mybir.dt.float32

_PATCHED = False


def _patch_act_tables():
    """Force exp+ln into the combined ACT table so only one table load is
    emitted (greedy per-function set choice otherwise alternates sets)."""
    global _PATCHED
    if _PATCHED:
        return
    import concourse.hw_specs as hw_specs
    real = hw_specs.get_activation_tables
    Exp = mybir.ActivationFunctionType.Exp
    Ln = mybir.ActivationFunctionType.Ln

    def patched(arch):
        out = {}
        for name, fns in dict(real(arch)).items():
            if name != "natural_log_exp_and_others":
                fns = fns - {Exp, Ln}
            out[name] = fns
        return out

    bacc.get_activation_tables = patched
    _PATCHED = True


def build_nc(f=F, finalize=True):
    """One core's shard: p01/p23 plane-pair tensors [P, 2f] bf16 (planes
    concatenated on the free axis), tgt [P, f] bf16, eye [P, P] bf16;
    out [P, 6] f32 = per-partition sums (p_t for c=0..3, lse half0, half1)."""
    _patch_act_tables()
    nc = bacc.Bacc("TRN2", target_bir_lowering=False, debug=False)
    p01 = nc.dram_tensor("p01", (P, 2 * f), BF16, kind="ExternalInput").ap()
    p23 = nc.dram_tensor("p23", (P, 2 * f), BF16, kind="ExternalInput").ap()
    tgt = nc.dram_tensor("tgt", (P, f), BF16, kind="ExternalInput").ap()
    eye = nc.dram_tensor("eye", (P, P), BF16, kind="ExternalInput").ap()
    outd = nc.dram_tensor("out", (P, 6), F32, kind="ExternalOutput").ap()

    Exp = mybir.ActivationFunctionType.Exp
    Ln = mybir.ActivationFunctionType.Ln
    h = f // 2  # PSUM bank half (512 f32 = one 2KB bank)

    es = ExitStack()
    sb = lambda name, shape, dt: es.enter_context(
        nc.sbuf_tensor(name, shape, dt)).ap()
    with nc.Block(name="ce") as block:
        s_p = [es.enter_context(nc.semaphore(f"s_p{i}")) for i in range(2)]
        s_eye = es.enter_context(nc.semaphore("s_eye"))
        s_tgt = es.enter_context(nc.semaphore("s_tgt"))
        s_e = es.enter_context(nc.semaphore("s_e"))
        s_mm = es.enter_context(nc.semaphore("s_mm"))
        s_res = es.enter_context(nc.semaphore("s_res"))
        s_out = es.enter_context(nc.semaphore("s_out"))

        pt = [sb(f"pt{i}", [P, 2 * f], BF16) for i in range(2)]
        tt = sb("tt", [P, f], BF16)
        te = sb("te", [P, P], BF16)
        e = [sb(f"e{i}", [P, 2 * f], BF16) for i in range(2)]
        scr = sb("scr", [P, 4 * f], BF16)
        lnout = sb("lnout", [P, f], BF16)
        res = sb("res", [P, 6], F32)
        dmy = sb("dmy", [P, 1], BF16)
        psum = es.enter_context(nc.psum_tensor("ps", [P, f], F32)).ap()

        @block.sync
        def _(sync):
            sync.dma_start(out=pt[0], in_=p01).then_inc(s_p[0], 16)
            sync.dma_start(out=tt, in_=tgt).then_inc(s_tgt, 16)
            sync.wait_ge(s_res, 6)  # 4 STT accums + 2 ln accums
            sync.dma_start(out=outd, in_=res).then_inc(s_out, 16)

        @block.gpsimd
        def _(gpsimd):
            gpsimd.dma_start(out=pt[1], in_=p23).then_inc(s_p[1], 16)
            gpsimd.dma_start(out=te, in_=eye).then_inc(s_eye, 16)

        @block.scalar
        def _(scalar):
            # Dummy activation first: the act-table-load pass places the
            # (1.3us) table DMA before it, so the table streams in parallel
            # with the input DMAs instead of serializing before exp0.
            scalar.activation(out=dmy, in_=dmy, func=Exp)
            for i in range(2):
                scalar.wait_ge(s_p[i], 16)
                scalar.activation(out=e[i], in_=pt[i], func=Exp).then_inc(s_e, 1)
            for half in range(2):
                lo = half * h
                scalar.wait_ge(s_mm, half + 1)
                scalar.activation(out=lnout[:, lo:lo + h], in_=psum[:, lo:lo + h],
                                  func=Ln,
                                  accum_out=res[:, 4 + half:5 + half],
                                  ).then_inc(s_res, 1)

        @block.tensor
        def _(tensor):
            tensor.wait_ge(s_eye, 16)
            # half-major on the last pair so bank0's group closes first and
            # ln_h0 overlaps bank1's matmuls.
            for i in range(2):
                tensor.wait_ge(s_e, i + 1)
                for half in range(2):
                    lo = half * h
                    for cc in range(2):   # plane within the pair
                        m = tensor.matmul(
                            psum[:, lo:lo + h], te,
                            e[i][:, cc * f + lo:cc * f + lo + h],
                            start=(i == 0 and cc == 0),
                            stop=(i == 1 and cc == 1))
                        if i == 1 and cc == 1:
                            m.then_inc(s_mm, 1)

        @block.vector
        def _(vector):
            vector.wait_ge(s_tgt, 16)
            for c in range(C):
                vector.wait_ge(s_p[c // 2], 16)
                vector.scalar_tensor_tensor(
                    out=scr[:, c * f:(c + 1) * f], in0=tt, scalar=float(c),
                    in1=pt[c // 2][:, (c % 2) * f:(c % 2) * f + f],
                    op0=mybir.AluOpType.is_equal, op1=mybir.AluOpType.mult,
                    accum_out=res[:, c:c + 1],
                ).then_inc(s_res, 1)

    es.close()
    if finalize:
        nc.finalize()
    return nc


_NC_CACHE = {}


def _get_nc(f=F):
    if f not in _NC_CACHE:
        _NC_CACHE[f] = build_nc(f)
    return _NC_CACHE[f]


def prep_inputs(preds, targets):
    """Host-side shard prep: S=-1 slice, per-channel planes, 8-way split."""
    p = np.asarray(preds)[:, -1]       # (N=4, C=4, 512, 512) f32
    t = np.asarray(targets)[:, -1]     # (4, 512, 512) int
    arr = np.transpose(p, (1, 0, 2, 3)).reshape(C, N_CORES, P, -1)
    arr = arr.astype(ml_dtypes.bfloat16)
    tf = t.reshape(N_CORES, P, -1).astype(ml_dtypes.bfloat16)
    ident = np.eye(P, dtype=ml_dtypes.bfloat16)
    maps = []
    for k in range(N_CORES):
        m = {
            "p01": np.ascontiguousarray(
                np.concatenate([arr[0, k], arr[1, k]], axis=1)),
            "p23": np.ascontiguousarray(
                np.concatenate([arr[2, k], arr[3, k]], axis=1)),
            "tgt": tf[k],
            "eye": ident,
        }
        maps.append(m)
    return maps


def reduce_outputs(results):
    total = 0.0
    for d in results:
        o = d["out"].astype(np.float64)
        total += float(o[:, 4:6].sum() - o[:, 0:4].sum())
    return np.float32(total / N_BATCH)


def kernel(preds, targets, _trace=False, _trace_kwargs=None):
    from concourse.bass_utils import run_bass_kernel_spmd

    in_maps = prep_inputs(preds, targets)
    f = in_maps[0]["tgt"].shape[1]
    nc = _get_nc(f=f)
    r = run_bass_kernel_spmd(
        nc, in_maps, core_ids=list(range(N_CORES)),
        trace=_trace, **(_trace_kwargs or {}),
    )
    kernel.last_run = r
    return reduce_outputs(r.results)


kernel.last_run = None
